# revision 12
# baseline (speedup 1.0000x reference)
"""Multi-head attention (RoPE, causal) Trainium2 Bass kernel.

Problem: nn_MultiHeadAttention_62431644615193
  x:     [2, 2048, 1024] f32
  mask:  [1, 1, 2048, 2048] i32 (causal tril expected)
  w_qkv: [1024, 3072] f32
  w_out: [1024, 1024] f32
  out:   [2, 2048, 1024] f32

Sharding over 8 cores: data-parallel on batch (2) x tensor-parallel on
heads (16 heads -> 4 per core). Each core computes a partial output
[2048, 1024] (its heads' contribution through w_out rows); the host sums
the 4 partials per batch.

Per-core dataflow (all matmuls float32r = full PE rate):
  1. qT,kT projection transposed:  qkT[c, s] = w_qk^T @ x^T   (c on partitions)
  2. v projection natural:          v[t, e]  = (x^T chunk as lhsT) @ w_v
  3. RoPE via small permutation matmul + DVE elementwise combine
  4. attention, scores transposed:  pT[t, s] = kT_blk^T-slice @ qT  (+ -1e9 mask
     matmul on diagonal blocks), ACT exp PSUM->SBUF, PV with ones-augmented V
     giving outT[e, s] rows 0-63 and the softmax denominator replicated on
     rows 64-127; normalize with reciprocal_approx_fast + tensor_mul
  5. out-projection: lhsT = attn_outT chunks, rhs = w_out rows for the core's
     heads -> partial [2048, 1024] streamed out per 128-row tile
"""

import math

import numpy as np

import concourse.bass as bass
import concourse.tile as tile
from concourse import bacc
import concourse.mybir as mybir
from concourse.bass_utils import run_bass_kernel_spmd

B, S, D = 2, 2048, 1024
H = 16
HD = D // H          # 64
HPC = H // 4         # 4 heads per core
ROPE_BASE = 10000.0

F32 = mybir.dt.float32
F32R = mybir.dt.float32r
AF = mybir.ActivationFunctionType

NEG = -1.0e9


# --------------------------------------------------------------------------
# bass program (shared by all 8 cores; per-core data differs)
# --------------------------------------------------------------------------

def build_nc(causal: bool = True):
    nc = bacc.Bacc("TRN2", target_bir_lowering=False, debug=False, num_devices=8)

    xT = nc.dram_tensor("xT", [D, S], F32, kind="ExternalInput")
    w_qk = nc.dram_tensor("w_qk", [D, 8 * HD], F32, kind="ExternalInput")
    w_v = nc.dram_tensor("w_v", [D, 4 * HD], F32, kind="ExternalInput")
    w_out = nc.dram_tensor("w_out", [4 * HD, D], F32, kind="ExternalInput")
    cos2 = nc.dram_tensor("cos2", [128, S], F32, kind="ExternalInput")
    sin2 = nc.dram_tensor("sin2", [128, S], F32, kind="ExternalInput")
    rotP = nc.dram_tensor("rotP", [128, 128], F32, kind="ExternalInput")
    ident = nc.dram_tensor("ident", [128, 128], F32, kind="ExternalInput")
    maskb = nc.dram_tensor("maskb", [128, 4 * 512], F32, kind="ExternalInput")
    onesd = nc.dram_tensor("onesd", [128, 64], F32, kind="ExternalInput")
    outp = nc.dram_tensor("outp", [S, D], F32, kind="ExternalOutput")

    NT = S // 128     # 16 t-blocks
    NI = 4            # fused quarter / attention s-chunks of 512

    with tile.TileContext(nc) as tc:
        with (
            tc.tile_pool(name="const", bufs=1) as cpool,
            tc.tile_pool(name="qkT", bufs=1) as qkTpool,
            tc.tile_pool(name="va", bufs=1) as vapool,
            tc.tile_pool(name="xq", bufs=2) as xqpool,
            tc.tile_pool(name="qkraw", bufs=8) as qkrawpool,
            tc.tile_pool(name="trig", bufs=2) as trigpool,
            tc.tile_pool(name="ropescratch", bufs=2) as rspool,
            tc.tile_pool(name="phat", bufs=3) as phatpool,
            tc.tile_pool(name="norm", bufs=3) as normpool,
            tc.tile_pool(name="attn_out", bufs=4) as aopool,
            tc.tile_pool(name="outstage", bufs=2) as ostpool,
            tc.tile_pool(name="ps", bufs=1, space="PSUM") as pspool,
        ):
            # ---------------- constants ----------------
            w_qk_t = [cpool.tile([128, 8 * HD], F32R, name=f"wqk{i}", tag=f"wqk{i}") for i in range(8)]
            w_v_t = [cpool.tile([128, 4 * HD], F32R, name=f"wv{i}", tag=f"wv{i}") for i in range(8)]
            w_out_t = [cpool.tile([128, D], F32R, name=f"wout{i}", tag=f"wout{i}") for i in range(2)]
            rotP_t = cpool.tile([128, 128], F32R)
            ident_t = cpool.tile([128, 128], F32R)
            maskb_t = cpool.tile([128, 4, 512], F32R)

            for dd in range(8):
                nc.sync.dma_start(
                    w_qk_t[dd][:], w_qk[128 * dd : 128 * dd + 128, :].bitcast(F32R)
                )
            for dd in range(8):
                nc.gpsimd.dma_start(
                    w_v_t[dd][:], w_v[128 * dd : 128 * dd + 128, :].bitcast(F32R)
                )
            for kk in range(2):
                nc.gpsimd.dma_start(
                    w_out_t[kk][:], w_out[128 * kk : 128 * kk + 128, :].bitcast(F32R)
                )
            nc.sync.dma_start(rotP_t[:], rotP[:].bitcast(F32R))
            nc.gpsimd.dma_start(ident_t[:], ident[:].bitcast(F32R))
            nc.gpsimd.dma_start(
                maskb_t[:], maskb[:].bitcast(F32R).rearrange("p (o s) -> p o s", o=4)
            )

            # v_aug storage: per (t-block j, head h): [v_h(64) | 1] = 65 cols
            va_t = vapool.tile([128, NT, 4, HD + 1], F32R)
            nc.gpsimd.dma_start(
                va_t[:, :, :, HD : HD + 1],
                onesd[:].bitcast(F32R).rearrange("p (j h c) -> p j h c", j=NT, h=4),
            )

            # final rotated qT/kT: tiles [q_h0;q_h1], [k_h0;k_h1], [q_h2;q_h3], [k_h2;k_h3]
            qkT = [qkTpool.tile([128, S], F32R, name=f"qkT{i}", tag=f"qkT{i}") for i in range(4)]

            # fused pipeline: per 512-wide chunk i: load+project+rope, then
            # attention chunk i (needs only t-blocks <= chunk end), then
            # out-projection for chunk i.
            for i in range(NI):
                s_sl = slice(512 * i, 512 * i + 512)

                # ---- loads for this quarter ----
                xq = xqpool.tile([128, 8, 512], F32R, tag="xq", name="xq")
                for dd in range(8):
                    nc.sync.dma_start(
                        xq[:, dd, :],
                        xT[128 * dd : 128 * dd + 128, s_sl].bitcast(F32R),
                    )
                cos_q = trigpool.tile([128, 512], F32, tag="cos", name="cos_q")
                sin_q = trigpool.tile([128, 512], F32, tag="sin", name="sin_q")
                nc.sync.dma_start(cos_q[:], cos2[:, s_sl])
                nc.sync.dma_start(sin_q[:], sin2[:, s_sl])

                # ---- qk projection + rope for this quarter ----
                for mt in range(4):
                    ps = pspool.tile([128, 512], F32, tag="mm1", bufs=2, name="ps")
                    for dd in range(8):
                        nc.tensor.matmul(
                            ps[:],
                            w_qk_t[dd][:, 128 * mt : 128 * mt + 128],
                            xq[:, dd, :],
                            start=(dd == 0),
                            stop=(dd == 7),
                        )
                    qk_raw = qkrawpool.tile([128, 512], F32R, tag="qkraw", name="qk_raw")
                    nc.vector.tensor_copy(qk_raw[:], ps[:])
                    psr = pspool.tile([128, 512], F32, tag="mm1", bufs=2, name="psr")
                    nc.tensor.matmul(psr[:], rotP_t[:], qk_raw[:], start=True, stop=True)
                    rotsin = rspool.tile([128, 512], F32, tag="rs", name="rotsin")
                    nc.vector.tensor_mul(rotsin[:], psr[:], sin_q[:])
                    qkcos = rspool.tile([128, 512], F32, tag="qkcos", name="qkcos")
                    nc.gpsimd.tensor_mul(qkcos[:], qk_raw[:].bitcast(F32), cos_q[:])
                    nc.vector.tensor_add(qkT[mt][:, s_sl], qkcos[:], rotsin[:])

                # ---- v projection for this quarter ----
                for st in range(4):
                    j = 4 * i + st
                    psv = pspool.tile([128, 4 * HD], F32, tag="mm1", bufs=2, name="psv")
                    for dd in range(8):
                        nc.tensor.matmul(
                            psv[:],
                            xq[:, dd, 128 * st : 128 * st + 128],
                            w_v_t[dd][:],
                            start=(dd == 0),
                            stop=(dd == 7),
                        )
                    nc.scalar.copy(
                        va_t[:, j, :, 0:HD], psv[:].rearrange("p (h c) -> p h c", h=4)
                    )

                # ---- attention for chunk i ----
                nblk = 4 * i + 4 if causal else NT
                ao = [
                    aopool.tile([128, 512], F32R, tag="aot", name=f"ao{hp}")
                    for hp in range(2)
                ]
                for hp in range(2):
                    qt = qkT[2 * hp]
                    kt = qkT[2 * hp + 1]
                    ps_pv_e = pspool.tile([HD + 1, 512], F32, tag="pv", bufs=2, name="ps_pv_e")
                    ps_pv_o = pspool.tile([HD + 1, 512], F32, tag="pv", bufs=2, name="ps_pv_o")
                    for j in range(nblk):
                        dvr = causal and 4 * i <= j
                        ps_qk = pspool.tile([128, 2, 512], F32, tag="mm2", bufs=2, name="ps_qk")
                        for sl2 in range(2):
                            hb = 64 * sl2
                            nc.tensor.matmul(
                                ps_qk[:, sl2, :],
                                kt[hb : hb + 64, 128 * j : 128 * j + 128],
                                qt[hb : hb + 64, s_sl],
                                start=True,
                                stop=not dvr,
                            )
                        if dvr:
                            o = j - 4 * i
                            for sl2 in range(2):
                                nc.tensor.matmul(
                                    ps_qk[:, sl2, :],
                                    ident_t[:],
                                    maskb_t[:, o, :],
                                    start=False,
                                    stop=True,
                                )
                        phat = phatpool.tile([128, 2, 512], F32R, tag="phat", name="phat")
                        nc.scalar.activation(
                            phat[:], ps_qk[:], AF.Exp, scale=1.0 / math.sqrt(HD)
                        )
                        nc.tensor.matmul(
                            ps_pv_e[:],
                            va_t[:, j, 2 * hp, :],
                            phat[:, 0, :],
                            start=(j == 0),
                            stop=(j == nblk - 1),
                        )
                        nc.tensor.matmul(
                            ps_pv_o[:],
                            va_t[:, j, 2 * hp + 1, :],
                            phat[:, 1, :],
                            start=(j == 0),
                            stop=(j == nblk - 1),
                        )
                    for sl2, ps_pv in ((0, ps_pv_e), (1, ps_pv_o)):
                        hb = 64 * sl2
                        den = normpool.tile([1, 512], F32, tag="den", name="den")
                        rec1 = normpool.tile([1, 512], F32, tag="rec1", name="rec1")
                        rec = normpool.tile([64, 512], F32, tag="rec", name="rec")
                        nc.scalar.copy(den[:], ps_pv[HD : HD + 1, :])
                        nc.vector.reciprocal_approx_fast(rec1[:], den[:])
                        nc.gpsimd.partition_broadcast(rec[:], rec1[:])
                        nc.vector.tensor_mul(
                            ao[hp][hb : hb + 64, :], ps_pv[0:HD, :], rec[:]
                        )

                # ---- out-projection for chunk i ----
                for st in range(4):
                    ssl = slice(512 * i + 128 * st, 512 * i + 128 * st + 128)
                    sloc = slice(128 * st, 128 * st + 128)
                    ostage = ostpool.tile([128, D], F32, tag="ost", name="ostage")
                    for n2 in range(2):
                        ps_o = pspool.tile([128, 512], F32, tag="mm2", bufs=2, name="ps_o")
                        for kk in range(2):
                            nc.tensor.matmul(
                                ps_o[:],
                                ao[kk][:, sloc],
                                w_out_t[kk][:, 512 * n2 : 512 * n2 + 512],
                                start=(kk == 0),
                                stop=(kk == 1),
                            )
                        nc.vector.tensor_copy(
                            ostage[:, 512 * n2 : 512 * n2 + 512], ps_o[:]
                        )
                    nc.scalar.dma_start(outp[ssl, :], ostage[:])

    nc.compile()
    return nc


# --------------------------------------------------------------------------
# host-side: constants, sharding, assembly
# --------------------------------------------------------------------------

def _rope_tables():
    inv_freq = 1.0 / (
        ROPE_BASE ** (np.arange(0, HD, 2, dtype=np.float32) / HD)
    )
    positions = np.arange(S, dtype=np.float32)
    freqs = np.outer(positions, inv_freq).astype(np.float32)     # [S, 32]
    emb = np.concatenate((freqs, freqs), axis=-1)                # [S, 64]
    cosT = np.cos(emb).T.astype(np.float32)                      # [64, S]
    sinT = np.sin(emb).T.astype(np.float32)
    cos2 = np.vstack([cosT, cosT]).copy()                        # [128, S]
    sin2 = np.vstack([sinT, sinT]).copy()
    return cos2, sin2


def _rot_lhsT():
    # rotate_half (interleaved): rot[2i] = -x[2i+1], rot[2i+1] = x[2i]
    # P[j, i]: rot[j] = sum_i P[j, i] x[i]; lhsT[i, j] = P[j, i]
    P = np.zeros((128, 128), np.float32)
    for base in (0, 64):
        for i2 in range(HD // 2):
            P[base + 2 * i2, base + 2 * i2 + 1] = -1.0
            P[base + 2 * i2 + 1, base + 2 * i2] = 1.0
    return P.T.copy()


def _mask_bias():
    # maskb[t, o, s'] = NEG where (t + 128*o) > s'  (within a 512 s-chunk,
    # for the 4 diagonal t-blocks at offsets o = j - 4i)
    t = np.arange(128)[:, None, None]
    o = np.arange(4)[None, :, None]
    sp = np.arange(512)[None, None, :]
    mb = np.where(t + 128 * o > sp, np.float32(NEG), np.float32(0.0))
    return mb.reshape(128, 4 * 512).astype(np.float32)


_CACHE: dict = {}


def _get_nc(causal: bool):
    key = ("nc", causal)
    if key not in _CACHE:
        _CACHE[key] = build_nc(causal)
    return _CACHE[key]


def _classify_mask(mask: np.ndarray) -> str:
    m = np.asarray(mask).reshape(S, S)
    if np.array_equal(m != 0, np.tril(np.ones((S, S), bool))):
        return "causal"
    if np.all(m != 0):
        return "full"
    return "other"


def make_in_maps(x, w_qkv, w_out):
    """Build the 8 per-core input dicts."""
    cos2, sin2 = _rope_tables()
    rotP = _rot_lhsT()
    maskb = _mask_bias()
    ident = np.eye(128, dtype=np.float32)
    onesd = np.ones((128, 64), np.float32)

    w3 = np.asarray(w_qkv).reshape(D, 3, H, HD)   # [D, {q,k,v}, H, hd]
    wo = np.asarray(w_out)                        # [D, D]; rows indexed [h, hd]
    xT = [np.ascontiguousarray(np.asarray(x)[b].T) for b in range(B)]  # [D, S]

    in_maps = []
    for c in range(8):
        b, hg = divmod(c, 4)
        hs = [4 * hg + i for i in range(HPC)]
        # w_qk cols: [q_h0, q_h1, k_h0, k_h1, q_h2, q_h3, k_h2, k_h3]
        wqk_cols = []
        for pair in range(2):
            for t in range(2):  # 0 = q, 1 = k
                for hh in (hs[2 * pair], hs[2 * pair + 1]):
                    wqk_cols.append(w3[:, t, hh, :])
        w_qk_c = np.ascontiguousarray(np.concatenate(wqk_cols, axis=1))  # [D, 512]
        w_v_c = np.ascontiguousarray(
            np.concatenate([w3[:, 2, hh, :] for hh in hs], axis=1)
        )  # [D, 256]
        w_out_c = np.ascontiguousarray(
            np.concatenate([wo[HD * hh : HD * hh + HD, :] for hh in hs], axis=0)
        )  # [256, D]
        in_maps.append(
            {
                "xT": xT[b],
                "w_qk": w_qk_c,
                "w_v": w_v_c,
                "w_out": w_out_c,
                "cos2": cos2,
                "sin2": sin2,
                "rotP": rotP,
                "ident": ident,
                "maskb": maskb,
                "onesd": onesd,
            }
        )
    return in_maps


def _reference_numpy(x, mask, w_qkv, w_out):
    """Exact fallback for non-causal, non-full masks (slow, host-side)."""
    x = np.asarray(x, np.float32)
    qkv = (x @ w_qkv).reshape(B, S, 3, H, HD)
    qkv = np.transpose(qkv, (2, 0, 3, 1, 4))
    q, k, v = qkv[0], qkv[1], qkv[2]
    cos2, sin2 = _rope_tables()
    cos = cos2[:HD].T[None, None]
    sin = sin2[:HD].T[None, None]

    def rot(t):
        t1 = t[..., ::2]
        t2 = t[..., 1::2]
        return np.stack((-t2, t1), axis=-1).reshape(t.shape)

    q = q * cos + rot(q) * sin
    k = k * cos + rot(k) * sin
    attn = np.einsum("bhsd,bhtd->bhst", q, k) / math.sqrt(HD)
    m = np.asarray(mask).reshape(1, 1, S, S)
    attn = np.where(m == 0, -np.inf, attn)
    attn = attn - attn.max(-1, keepdims=True)
    np.exp(attn, out=attn)
    attn /= attn.sum(-1, keepdims=True)
    out = np.einsum("bhst,bhtd->bhsd", attn, v)
    out = np.transpose(out, (0, 2, 1, 3)).reshape(B, S, D)
    return (out @ w_out).astype(np.float32)


class Runner:
    """Cached jitted SPMD runner (mirrors bass2jax.run_bass_via_pjrt)."""

    def __init__(self, nc, n_cores: int = 8):
        import jax
        import concourse.mybir as _mybir
        from concourse import bass2jax
        from jax.experimental.shard_map import shard_map
        from jax.sharding import Mesh, PartitionSpec

        bass2jax.install_neuronx_cc_hook()
        self.jax = jax
        self.n_cores = n_cores
        in_names, out_names, out_avals, zero_outs = [], [], [], []
        for alloc in nc.m.functions[0].allocations:
            if not isinstance(alloc, _mybir.MemoryLocationSet):
                continue
            name = alloc.memorylocations[0].name
            if alloc.kind == "ExternalInput":
                in_names.append(name)
            elif alloc.kind == "ExternalOutput":
                out_names.append(name)
                shape = tuple(alloc.tensor_shape)
                dtype = _mybir.dt.np(alloc.dtype)
                out_avals.append(jax.core.ShapedArray(shape, dtype))
                zero_outs.append(np.zeros(shape, dtype))
        self.in_names = list(in_names)
        self.out_names = out_names
        self.out_avals = out_avals
        self.zero_outs = zero_outs
        all_names = in_names + out_names

        def _body(*args):
            outs = bass2jax._bass_exec_p.bind(
                *args,
                out_avals=tuple(out_avals),
                in_names=tuple(all_names),
                out_names=tuple(out_names),
                lowering_input_output_aliases=(),
                sim_require_finite=True,
                sim_require_nnan=True,
                nc=nc,
            )
            return tuple(outs)

        devices = jax.devices()[:n_cores]
        self.mesh = Mesh(np.asarray(devices), ("core",))
        n_args = len(all_names)
        self.sharded = jax.jit(
            shard_map(
                _body,
                mesh=self.mesh,
                in_specs=(PartitionSpec("core"),) * n_args,
                out_specs=(PartitionSpec("core"),) * len(out_names),
                check_rep=False,
            )
        )

    def concat_inputs(self, in_maps):
        cols = []
        for name in self.in_names:
            if name == "partition_id":
                cols.append(
                    np.arange(self.n_cores, dtype=np.uint32).reshape(
                        self.n_cores, 1
                    )
                )
            else:
                cols.append(
                    np.concatenate([np.asarray(m[name]) for m in in_maps], axis=0)
                )
        return cols

    def device_put(self, concat_in):
        """Place concatenated inputs (and zero output buffers) on the mesh."""
        from jax.sharding import NamedSharding, PartitionSpec

        sh = NamedSharding(self.mesh, PartitionSpec("core"))
        args = concat_in + [
            np.zeros((self.n_cores * z.shape[0], *z.shape[1:]), z.dtype)
            for z in self.zero_outs
        ]
        return [self.jax.device_put(a, sh) for a in args]

    def run_dev(self, dev_args):
        return self.sharded(*dev_args)

    def run(self, in_maps):
        dev_args = self.device_put(self.concat_inputs(in_maps))
        out_arrs = self.sharded(*dev_args)
        outs = []
        for c in range(self.n_cores):
            outs.append(
                {
                    name: np.asarray(out_arrs[i]).reshape(
                        self.n_cores, *self.out_avals[i].shape
                    )[c]
                    for i, name in enumerate(self.out_names)
                }
            )
        return outs


def _get_runner(causal: bool) -> Runner:
    key = ("runner", causal)
    if key not in _CACHE:
        _CACHE[key] = Runner(_get_nc(causal))
    return _CACHE[key]


def run_spmd(in_maps, causal: bool = True, **kw):
    nc = _get_nc(causal)
    return run_bass_kernel_spmd(nc, in_maps, core_ids=list(range(8)), **kw)


def kernel(x, mask, w_qkv, w_out):
    kind = _classify_mask(mask)
    if kind == "other":
        return _reference_numpy(x, mask, w_qkv, w_out)
    in_maps = make_in_maps(x, w_qkv, w_out)
    results = _get_runner(causal=(kind == "causal")).run(in_maps)
    out = np.zeros((B, S, D), np.float32)
    for c in range(8):
        out[c // 4] += results[c]["outp"]
    return out


if __name__ == "__main__":
    rng = np.random.default_rng(0)
    x = rng.standard_normal((B, S, D)).astype(np.float32)
    mask = np.tril(np.ones((S, S), np.int32)).reshape(1, 1, S, S)
    w_qkv = (rng.standard_normal((D, 3 * D)) * 0.02).astype(np.float32)
    w_out = (rng.standard_normal((D, D)) * 0.02).astype(np.float32)
    got = kernel(x, mask, w_qkv, w_out)
    print("kernel ran, out shape", got.shape)


# revision 13
# speedup vs baseline: 11.6575x; 11.6575x over previous
"""Multi-head attention (RoPE, causal) Trainium2 Bass kernel.

Problem: nn_MultiHeadAttention_62431644615193
  x:     [2, 2048, 1024] f32
  mask:  [1, 1, 2048, 2048] i32 (causal tril expected)
  w_qkv: [1024, 3072] f32
  w_out: [1024, 1024] f32
  out:   [2, 2048, 1024] f32

Sharding over 8 cores: data-parallel on batch (2) x tensor-parallel on
heads (16 heads -> 4 per core). Each core computes a partial output
[2048, 1024] (its heads' contribution through w_out rows); the host sums
the 4 partials per batch.

Per-core dataflow (all matmuls float32r = full PE rate):
  1. qT,kT projection transposed:  qkT[c, s] = w_qk^T @ x^T   (c on partitions)
  2. v projection natural:          v[t, e]  = (x^T chunk as lhsT) @ w_v
  3. RoPE via small permutation matmul + DVE elementwise combine
  4. attention, scores transposed:  pT[t, s] = kT_blk^T-slice @ qT  (+ -1e9 mask
     matmul on diagonal blocks), ACT exp PSUM->SBUF, PV with ones-augmented V
     giving outT[e, s] rows 0-63 and the softmax denominator replicated on
     rows 64-127; normalize with reciprocal_approx_fast + tensor_mul
  5. out-projection: lhsT = attn_outT chunks, rhs = w_out rows for the core's
     heads -> partial [2048, 1024] streamed out per 128-row tile
"""

import math

import numpy as np

import concourse.bass as bass
import concourse.tile as tile
from concourse import bacc
import concourse.mybir as mybir
from concourse.bass_utils import run_bass_kernel_spmd

B, S, D = 2, 2048, 1024
H = 16
HD = D // H          # 64
HPC = H // 4         # 4 heads per core
ROPE_BASE = 10000.0

F32 = mybir.dt.float32
F32R = mybir.dt.float32r
AF = mybir.ActivationFunctionType

NEG = -1.0e9


# --------------------------------------------------------------------------
# bass program (shared by all 8 cores; per-core data differs)
# --------------------------------------------------------------------------

def build_nc(causal: bool = True):
    nc = bacc.Bacc("TRN2", target_bir_lowering=False, debug=False, num_devices=8)

    xT = nc.dram_tensor("xT", [D, S], F32, kind="ExternalInput")
    w_qk = nc.dram_tensor("w_qk", [D, 8 * HD], F32, kind="ExternalInput")
    w_v = nc.dram_tensor("w_v", [D, 4 * HD], F32, kind="ExternalInput")
    w_out = nc.dram_tensor("w_out", [4 * HD, D], F32, kind="ExternalInput")
    cos2 = nc.dram_tensor("cos2", [128, S], F32, kind="ExternalInput")
    sin2 = nc.dram_tensor("sin2", [128, S], F32, kind="ExternalInput")
    rotP = nc.dram_tensor("rotP", [128, 128], F32, kind="ExternalInput")
    ident = nc.dram_tensor("ident", [128, 128], F32, kind="ExternalInput")
    maskb = nc.dram_tensor("maskb", [128, 4 * 512], F32, kind="ExternalInput")
    onesd = nc.dram_tensor("onesd", [128, 64], F32, kind="ExternalInput")
    outp = nc.dram_tensor("outp", [S, D], F32, kind="ExternalOutput")

    NT = S // 128     # 16 t-blocks
    NI = 4            # fused quarter / attention s-chunks of 512

    with tile.TileContext(nc) as tc:
        with (
            tc.tile_pool(name="const", bufs=1) as cpool,
            tc.tile_pool(name="qkT", bufs=1) as qkTpool,
            tc.tile_pool(name="va", bufs=1) as vapool,
            tc.tile_pool(name="xq", bufs=2) as xqpool,
            tc.tile_pool(name="qkraw", bufs=8) as qkrawpool,
            tc.tile_pool(name="trig", bufs=2) as trigpool,
            tc.tile_pool(name="ropescratch", bufs=2) as rspool,
            tc.tile_pool(name="phat", bufs=3) as phatpool,
            tc.tile_pool(name="norm", bufs=3) as normpool,
            tc.tile_pool(name="attn_out", bufs=4) as aopool,
            tc.tile_pool(name="outstage", bufs=2) as ostpool,
            tc.tile_pool(name="ps", bufs=1, space="PSUM") as pspool,
        ):
            # ---------------- constants ----------------
            w_qk_t = [cpool.tile([128, 8 * HD], F32R, name=f"wqk{i}", tag=f"wqk{i}") for i in range(8)]
            w_v_t = [cpool.tile([128, 4 * HD], F32R, name=f"wv{i}", tag=f"wv{i}") for i in range(8)]
            w_out_t = [cpool.tile([128, D], F32R, name=f"wout{i}", tag=f"wout{i}") for i in range(2)]
            rotP_t = cpool.tile([128, 128], F32R)
            ident_t = cpool.tile([128, 128], F32R)
            maskb_t = cpool.tile([128, 4, 512], F32R)

            for dd in range(8):
                nc.sync.dma_start(
                    w_qk_t[dd][:], w_qk[128 * dd : 128 * dd + 128, :].bitcast(F32R)
                )
            for dd in range(8):
                nc.gpsimd.dma_start(
                    w_v_t[dd][:], w_v[128 * dd : 128 * dd + 128, :].bitcast(F32R)
                )
            for kk in range(2):
                nc.gpsimd.dma_start(
                    w_out_t[kk][:], w_out[128 * kk : 128 * kk + 128, :].bitcast(F32R)
                )
            nc.sync.dma_start(rotP_t[:], rotP[:].bitcast(F32R))
            nc.gpsimd.dma_start(ident_t[:], ident[:].bitcast(F32R))
            nc.gpsimd.dma_start(
                maskb_t[:], maskb[:].bitcast(F32R).rearrange("p (o s) -> p o s", o=4)
            )

            # v_aug storage: per (t-block j, head h): [v_h(64) | 1] = 65 cols
            va_t = vapool.tile([128, NT, 4, HD + 1], F32R)
            nc.gpsimd.dma_start(
                va_t[:, :, :, HD : HD + 1],
                onesd[:].bitcast(F32R).rearrange("p (j h c) -> p j h c", j=NT, h=4),
            )

            # final rotated qT/kT: tiles [q_h0;q_h1], [k_h0;k_h1], [q_h2;q_h3], [k_h2;k_h3]
            qkT = [qkTpool.tile([128, S], F32R, name=f"qkT{i}", tag=f"qkT{i}") for i in range(4)]

            # fused pipeline: per 512-wide chunk i: load+project+rope, then
            # attention chunk i (needs only t-blocks <= chunk end), then
            # out-projection for chunk i.
            for i in range(NI):
                s_sl = slice(512 * i, 512 * i + 512)

                # ---- loads for this quarter ----
                xq = xqpool.tile([128, 8, 512], F32R, tag="xq", name="xq")
                for dd in range(8):
                    nc.sync.dma_start(
                        xq[:, dd, :],
                        xT[128 * dd : 128 * dd + 128, s_sl].bitcast(F32R),
                    )
                cos_q = trigpool.tile([128, 512], F32, tag="cos", name="cos_q")
                sin_q = trigpool.tile([128, 512], F32, tag="sin", name="sin_q")
                nc.sync.dma_start(cos_q[:], cos2[:, s_sl])
                nc.sync.dma_start(sin_q[:], sin2[:, s_sl])

                # ---- qk projection + rope for this quarter ----
                for mt in range(4):
                    ps = pspool.tile([128, 512], F32, tag="mm1", bufs=2, name="ps")
                    for dd in range(8):
                        nc.tensor.matmul(
                            ps[:],
                            w_qk_t[dd][:, 128 * mt : 128 * mt + 128],
                            xq[:, dd, :],
                            start=(dd == 0),
                            stop=(dd == 7),
                        )
                    qk_raw = qkrawpool.tile([128, 512], F32R, tag="qkraw", name="qk_raw")
                    nc.vector.tensor_copy(qk_raw[:], ps[:])
                    psr = pspool.tile([128, 512], F32, tag="mm1", bufs=2, name="psr")
                    nc.tensor.matmul(psr[:], rotP_t[:], qk_raw[:], start=True, stop=True)
                    rotsin = rspool.tile([128, 512], F32, tag="rs", name="rotsin")
                    nc.vector.tensor_mul(rotsin[:], psr[:], sin_q[:])
                    qkcos = rspool.tile([128, 512], F32, tag="qkcos", name="qkcos")
                    nc.gpsimd.tensor_mul(qkcos[:], qk_raw[:].bitcast(F32), cos_q[:])
                    nc.vector.tensor_add(qkT[mt][:, s_sl], qkcos[:], rotsin[:])

                # ---- v projection for this quarter ----
                for st in range(4):
                    j = 4 * i + st
                    psv = pspool.tile([128, 4 * HD], F32, tag="mm1", bufs=2, name="psv")
                    for dd in range(8):
                        nc.tensor.matmul(
                            psv[:],
                            xq[:, dd, 128 * st : 128 * st + 128],
                            w_v_t[dd][:],
                            start=(dd == 0),
                            stop=(dd == 7),
                        )
                    nc.scalar.copy(
                        va_t[:, j, :, 0:HD], psv[:].rearrange("p (h c) -> p h c", h=4)
                    )

                # ---- attention for chunk i ----
                nblk = 4 * i + 4 if causal else NT
                ao = [
                    aopool.tile([128, 512], F32R, tag="aot", name=f"ao{hp}")
                    for hp in range(2)
                ]
                for hp in range(2):
                    qt = qkT[2 * hp]
                    kt = qkT[2 * hp + 1]
                    ps_pv_e = pspool.tile([HD + 1, 512], F32, tag="pv", bufs=2, name="ps_pv_e")
                    ps_pv_o = pspool.tile([HD + 1, 512], F32, tag="pv", bufs=2, name="ps_pv_o")
                    for j in range(nblk):
                        dvr = causal and 4 * i <= j
                        ps_qk = pspool.tile([128, 2, 512], F32, tag="mm2", bufs=2, name="ps_qk")
                        for sl2 in range(2):
                            hb = 64 * sl2
                            nc.tensor.matmul(
                                ps_qk[:, sl2, :],
                                kt[hb : hb + 64, 128 * j : 128 * j + 128],
                                qt[hb : hb + 64, s_sl],
                                start=True,
                                stop=not dvr,
                            )
                        if dvr:
                            o = j - 4 * i
                            for sl2 in range(2):
                                nc.tensor.matmul(
                                    ps_qk[:, sl2, :],
                                    ident_t[:],
                                    maskb_t[:, o, :],
                                    start=False,
                                    stop=True,
                                )
                        phat = phatpool.tile([128, 2, 512], F32R, tag="phat", name="phat")
                        nc.scalar.activation(
                            phat[:], ps_qk[:], AF.Exp, scale=1.0 / math.sqrt(HD)
                        )
                        nc.tensor.matmul(
                            ps_pv_e[:],
                            va_t[:, j, 2 * hp, :],
                            phat[:, 0, :],
                            start=(j == 0),
                            stop=(j == nblk - 1),
                        )
                        nc.tensor.matmul(
                            ps_pv_o[:],
                            va_t[:, j, 2 * hp + 1, :],
                            phat[:, 1, :],
                            start=(j == 0),
                            stop=(j == nblk - 1),
                        )
                    for sl2, ps_pv in ((0, ps_pv_e), (1, ps_pv_o)):
                        hb = 64 * sl2
                        den = normpool.tile([1, 512], F32, tag="den", name="den")
                        rec1 = normpool.tile([1, 512], F32, tag="rec1", name="rec1")
                        rec = normpool.tile([64, 512], F32, tag="rec", name="rec")
                        nc.scalar.copy(den[:], ps_pv[HD : HD + 1, :])
                        nc.vector.reciprocal_approx_fast(rec1[:], den[:])
                        nc.gpsimd.partition_broadcast(rec[:], rec1[:])
                        nc.vector.tensor_mul(
                            ao[hp][hb : hb + 64, :], ps_pv[0:HD, :], rec[:]
                        )

                # ---- out-projection for chunk i ----
                for st in range(4):
                    ssl = slice(512 * i + 128 * st, 512 * i + 128 * st + 128)
                    sloc = slice(128 * st, 128 * st + 128)
                    ostage = ostpool.tile([128, D], F32, tag="ost", name="ostage")
                    for n2 in range(2):
                        ps_o = pspool.tile([128, 512], F32, tag="mm2", bufs=2, name="ps_o")
                        for kk in range(2):
                            nc.tensor.matmul(
                                ps_o[:],
                                ao[kk][:, sloc],
                                w_out_t[kk][:, 512 * n2 : 512 * n2 + 512],
                                start=(kk == 0),
                                stop=(kk == 1),
                            )
                        nc.vector.tensor_copy(
                            ostage[:, 512 * n2 : 512 * n2 + 512], ps_o[:]
                        )
                    nc.scalar.dma_start(outp[ssl, :], ostage[:])

    nc.compile()
    return nc


# --------------------------------------------------------------------------
# host-side: constants, sharding, assembly
# --------------------------------------------------------------------------

def _rope_tables():
    inv_freq = 1.0 / (
        ROPE_BASE ** (np.arange(0, HD, 2, dtype=np.float32) / HD)
    )
    positions = np.arange(S, dtype=np.float32)
    freqs = np.outer(positions, inv_freq).astype(np.float32)     # [S, 32]
    emb = np.concatenate((freqs, freqs), axis=-1)                # [S, 64]
    cosT = np.cos(emb).T.astype(np.float32)                      # [64, S]
    sinT = np.sin(emb).T.astype(np.float32)
    cos2 = np.vstack([cosT, cosT]).copy()                        # [128, S]
    sin2 = np.vstack([sinT, sinT]).copy()
    return cos2, sin2


def _rot_lhsT():
    # rotate_half (interleaved): rot[2i] = -x[2i+1], rot[2i+1] = x[2i]
    # P[j, i]: rot[j] = sum_i P[j, i] x[i]; lhsT[i, j] = P[j, i]
    P = np.zeros((128, 128), np.float32)
    for base in (0, 64):
        for i2 in range(HD // 2):
            P[base + 2 * i2, base + 2 * i2 + 1] = -1.0
            P[base + 2 * i2 + 1, base + 2 * i2] = 1.0
    return P.T.copy()


def _mask_bias():
    # maskb[t, o, s'] = NEG where (t + 128*o) > s'  (within a 512 s-chunk,
    # for the 4 diagonal t-blocks at offsets o = j - 4i)
    t = np.arange(128)[:, None, None]
    o = np.arange(4)[None, :, None]
    sp = np.arange(512)[None, None, :]
    mb = np.where(t + 128 * o > sp, np.float32(NEG), np.float32(0.0))
    return mb.reshape(128, 4 * 512).astype(np.float32)


_CACHE: dict = {}


def _get_nc(causal: bool):
    key = ("nc", causal)
    if key not in _CACHE:
        _CACHE[key] = build_nc(causal)
    return _CACHE[key]


def _classify_mask(mask: np.ndarray) -> str:
    m = np.asarray(mask).reshape(S, S)
    if np.array_equal(m != 0, np.tril(np.ones((S, S), bool))):
        return "causal"
    if np.all(m != 0):
        return "full"
    return "other"


def make_in_maps(x, w_qkv, w_out):
    """Build the 8 per-core input dicts."""
    cos2, sin2 = _rope_tables()
    rotP = _rot_lhsT()
    maskb = _mask_bias()
    ident = np.eye(128, dtype=np.float32)
    onesd = np.ones((128, 64), np.float32)

    w3 = np.asarray(w_qkv).reshape(D, 3, H, HD)   # [D, {q,k,v}, H, hd]
    wo = np.asarray(w_out)                        # [D, D]; rows indexed [h, hd]
    xT = [np.ascontiguousarray(np.asarray(x)[b].T) for b in range(B)]  # [D, S]

    in_maps = []
    for c in range(8):
        b, hg = divmod(c, 4)
        hs = [4 * hg + i for i in range(HPC)]
        # w_qk cols: [q_h0, q_h1, k_h0, k_h1, q_h2, q_h3, k_h2, k_h3]
        wqk_cols = []
        for pair in range(2):
            for t in range(2):  # 0 = q, 1 = k
                for hh in (hs[2 * pair], hs[2 * pair + 1]):
                    wqk_cols.append(w3[:, t, hh, :])
        w_qk_c = np.ascontiguousarray(np.concatenate(wqk_cols, axis=1))  # [D, 512]
        w_v_c = np.ascontiguousarray(
            np.concatenate([w3[:, 2, hh, :] for hh in hs], axis=1)
        )  # [D, 256]
        w_out_c = np.ascontiguousarray(
            np.concatenate([wo[HD * hh : HD * hh + HD, :] for hh in hs], axis=0)
        )  # [256, D]
        in_maps.append(
            {
                "xT": xT[b],
                "w_qk": w_qk_c,
                "w_v": w_v_c,
                "w_out": w_out_c,
                "cos2": cos2,
                "sin2": sin2,
                "rotP": rotP,
                "ident": ident,
                "maskb": maskb,
                "onesd": onesd,
            }
        )
    return in_maps


def _reference_numpy(x, mask, w_qkv, w_out):
    """Exact fallback for non-causal, non-full masks (slow, host-side)."""
    x = np.asarray(x, np.float32)
    qkv = (x @ w_qkv).reshape(B, S, 3, H, HD)
    qkv = np.transpose(qkv, (2, 0, 3, 1, 4))
    q, k, v = qkv[0], qkv[1], qkv[2]
    cos2, sin2 = _rope_tables()
    cos = cos2[:HD].T[None, None]
    sin = sin2[:HD].T[None, None]

    def rot(t):
        t1 = t[..., ::2]
        t2 = t[..., 1::2]
        return np.stack((-t2, t1), axis=-1).reshape(t.shape)

    q = q * cos + rot(q) * sin
    k = k * cos + rot(k) * sin
    attn = np.einsum("bhsd,bhtd->bhst", q, k) / math.sqrt(HD)
    m = np.asarray(mask).reshape(1, 1, S, S)
    attn = np.where(m == 0, -np.inf, attn)
    attn = attn - attn.max(-1, keepdims=True)
    np.exp(attn, out=attn)
    attn /= attn.sum(-1, keepdims=True)
    out = np.einsum("bhst,bhtd->bhsd", attn, v)
    out = np.transpose(out, (0, 2, 1, 3)).reshape(B, S, D)
    return (out @ w_out).astype(np.float32)


class Runner:
    """Cached jitted SPMD runner (mirrors bass2jax.run_bass_via_pjrt)."""

    def __init__(self, nc, n_cores: int = 8):
        import jax
        import concourse.mybir as _mybir
        from concourse import bass2jax
        from jax.experimental.shard_map import shard_map
        from jax.sharding import Mesh, PartitionSpec

        bass2jax.install_neuronx_cc_hook()
        self.jax = jax
        self.n_cores = n_cores
        self._nc = nc
        in_names, out_names, out_avals, zero_outs = [], [], [], []
        for alloc in nc.m.functions[0].allocations:
            if not isinstance(alloc, _mybir.MemoryLocationSet):
                continue
            name = alloc.memorylocations[0].name
            if alloc.kind == "ExternalInput":
                in_names.append(name)
            elif alloc.kind == "ExternalOutput":
                out_names.append(name)
                shape = tuple(alloc.tensor_shape)
                dtype = _mybir.dt.np(alloc.dtype)
                out_avals.append(jax.core.ShapedArray(shape, dtype))
                zero_outs.append(np.zeros(shape, dtype))
        self.in_names = list(in_names)
        self.out_names = out_names
        self.out_avals = out_avals
        self.zero_outs = zero_outs
        all_names = in_names + out_names

        def _body(*args):
            outs = bass2jax._bass_exec_p.bind(
                *args,
                out_avals=tuple(out_avals),
                in_names=tuple(all_names),
                out_names=tuple(out_names),
                lowering_input_output_aliases=(),
                sim_require_finite=True,
                sim_require_nnan=True,
                nc=nc,
            )
            return tuple(outs)

        devices = jax.devices()[:n_cores]
        self.mesh = Mesh(np.asarray(devices), ("core",))
        n_args = len(all_names)
        self.sharded = jax.jit(
            shard_map(
                _body,
                mesh=self.mesh,
                in_specs=(PartitionSpec("core"),) * n_args,
                out_specs=(PartitionSpec("core"),) * len(out_names),
                check_rep=False,
            )
        )

    def concat_inputs(self, in_maps):
        cols = []
        for name in self.in_names:
            if name == "partition_id":
                cols.append(
                    np.arange(self.n_cores, dtype=np.uint32).reshape(
                        self.n_cores, 1
                    )
                )
            else:
                cols.append(
                    np.concatenate([np.asarray(m[name]) for m in in_maps], axis=0)
                )
        return cols

    def device_put(self, concat_in):
        """Place concatenated inputs (and zero output buffers) on the mesh."""
        from jax.sharding import NamedSharding, PartitionSpec

        sh = NamedSharding(self.mesh, PartitionSpec("core"))
        args = concat_in + [
            np.zeros((self.n_cores * z.shape[0], *z.shape[1:]), z.dtype)
            for z in self.zero_outs
        ]
        return [self.jax.device_put(a, sh) for a in args]

    def run_dev(self, dev_args):
        return self.sharded(*dev_args)

    def make_bench(self, n_reps: int):
        """Jitted fn executing the NEFF n_reps times serially on-device."""
        import jax
        from concourse import bass2jax
        from jax.experimental.shard_map import shard_map
        from jax.sharding import Mesh, PartitionSpec

        nc = self._nc
        out_avals = self.out_avals
        all_names = self.in_names + self.out_names
        out_names = self.out_names

        def _body(*args):
            outs = None
            for _ in range(n_reps):
                outs = bass2jax._bass_exec_p.bind(
                    *args,
                    out_avals=tuple(out_avals),
                    in_names=tuple(all_names),
                    out_names=tuple(out_names),
                    lowering_input_output_aliases=(),
                    sim_require_finite=True,
                    sim_require_nnan=True,
                    nc=nc,
                )
            return tuple(outs)

        n_args = len(all_names)
        return jax.jit(
            shard_map(
                _body,
                mesh=self.mesh,
                in_specs=(PartitionSpec("core"),) * n_args,
                out_specs=(PartitionSpec("core"),) * len(out_names),
                check_rep=False,
            )
        )

    def run(self, in_maps):
        dev_args = self.device_put(self.concat_inputs(in_maps))
        out_arrs = self.sharded(*dev_args)
        outs = []
        for c in range(self.n_cores):
            outs.append(
                {
                    name: np.asarray(out_arrs[i]).reshape(
                        self.n_cores, *self.out_avals[i].shape
                    )[c]
                    for i, name in enumerate(self.out_names)
                }
            )
        return outs


def _get_runner(causal: bool) -> Runner:
    key = ("runner", causal)
    if key not in _CACHE:
        _CACHE[key] = Runner(_get_nc(causal))
    return _CACHE[key]


def run_spmd(in_maps, causal: bool = True, **kw):
    nc = _get_nc(causal)
    return run_bass_kernel_spmd(nc, in_maps, core_ids=list(range(8)), **kw)


def kernel(x, mask, w_qkv, w_out):
    kind = _classify_mask(mask)
    if kind == "other":
        return _reference_numpy(x, mask, w_qkv, w_out)
    in_maps = make_in_maps(x, w_qkv, w_out)
    results = _get_runner(causal=(kind == "causal")).run(in_maps)
    out = np.zeros((B, S, D), np.float32)
    for c in range(8):
        out[c // 4] += results[c]["outp"]
    return out


if __name__ == "__main__":
    rng = np.random.default_rng(0)
    x = rng.standard_normal((B, S, D)).astype(np.float32)
    mask = np.tril(np.ones((S, S), np.int32)).reshape(1, 1, S, S)
    w_qkv = (rng.standard_normal((D, 3 * D)) * 0.02).astype(np.float32)
    w_out = (rng.standard_normal((D, D)) * 0.02).astype(np.float32)
    got = kernel(x, mask, w_qkv, w_out)
    print("kernel ran, out shape", got.shape)


# revision 25
# speedup vs baseline: 455.3266x; 39.0585x over previous
"""Multi-head attention (RoPE, causal) Trainium2 Bass kernel.

Problem: nn_MultiHeadAttention_62431644615193
  x:     [2, 2048, 1024] f32
  mask:  [1, 1, 2048, 2048] i32 (causal tril expected)
  w_qkv: [1024, 3072] f32
  w_out: [1024, 1024] f32
  out:   [2, 2048, 1024] f32

Sharding over 8 cores: data-parallel on batch (2) x tensor-parallel on
heads (16 heads -> 4 per core). Each core computes a partial output
[2048, 1024] (its heads' contribution through w_out rows); the host sums
the 4 partials per batch.

Per-core dataflow (all matmuls float32r = full PE rate):
  1. qT,kT projection transposed:  qkT[c, s] = w_qk^T @ x^T   (c on partitions)
  2. v projection natural:          v[t, e]  = (x^T chunk as lhsT) @ w_v
  3. RoPE via small permutation matmul + DVE elementwise combine
  4. attention, scores transposed:  pT[t, s] = kT_blk^T-slice @ qT  (+ -1e9 mask
     matmul on diagonal blocks), ACT exp PSUM->SBUF, PV with ones-augmented V
     giving outT[e, s] rows 0-63 and the softmax denominator replicated on
     rows 64-127; normalize with reciprocal_approx_fast + tensor_mul
  5. out-projection: lhsT = attn_outT chunks, rhs = w_out rows for the core's
     heads -> partial [2048, 1024] streamed out per 128-row tile
"""

import math

import numpy as np

import concourse.bass as bass
import concourse.tile as tile
from concourse import bacc
import concourse.mybir as mybir
from concourse.bass_utils import run_bass_kernel_spmd

B, S, D = 2, 2048, 1024
H = 16
HD = D // H          # 64
HPC = H // 4         # 4 heads per core
ROPE_BASE = 10000.0

F32 = mybir.dt.float32
F32R = mybir.dt.float32r
AF = mybir.ActivationFunctionType

NEG = -1.0e9


# --------------------------------------------------------------------------
# bass program (shared by all 8 cores; per-core data differs)
# --------------------------------------------------------------------------

def build_nc(causal: bool = True, reps: int = 1):
    nc = bacc.Bacc("TRN2", target_bir_lowering=False, debug=False, num_devices=8)

    xT = nc.dram_tensor("xT", [D, S], F32, kind="ExternalInput")
    w_qk = nc.dram_tensor("w_qk", [D, 8 * HD], F32, kind="ExternalInput")
    w_v = nc.dram_tensor("w_v", [D, 4 * HD], F32, kind="ExternalInput")
    w_out = nc.dram_tensor("w_out", [4 * HD, D], F32, kind="ExternalInput")
    cos2 = nc.dram_tensor("cos2", [128, S], F32, kind="ExternalInput")
    sin2 = nc.dram_tensor("sin2", [128, S], F32, kind="ExternalInput")
    rotP = nc.dram_tensor("rotP", [128, 128], F32, kind="ExternalInput")
    ident = nc.dram_tensor("ident", [128, 128], F32, kind="ExternalInput")
    maskb = nc.dram_tensor("maskb", [128, 4 * 512], F32, kind="ExternalInput")
    onesd = nc.dram_tensor("onesd", [128, 64], F32, kind="ExternalInput")
    outp = nc.dram_tensor("outp", [S, D], F32, kind="ExternalOutput")

    NT = S // 128     # 16 t-blocks
    NI = 4            # fused quarter / attention s-chunks of 512

    with tile.TileContext(nc) as tc:
        with (
            tc.tile_pool(name="const", bufs=1) as cpool,
            tc.tile_pool(name="qkT", bufs=1) as qkTpool,
            tc.tile_pool(name="va", bufs=1) as vapool,
            tc.tile_pool(name="xq", bufs=2) as xqpool,
            tc.tile_pool(name="qkraw", bufs=8) as qkrawpool,
            tc.tile_pool(name="trig", bufs=2) as trigpool,
            tc.tile_pool(name="ropescratch", bufs=2) as rspool,
            tc.tile_pool(name="phat", bufs=4) as phatpool,
            tc.tile_pool(name="norm", bufs=3) as normpool,
            tc.tile_pool(name="attn_out", bufs=4) as aopool,
            tc.tile_pool(name="outstage", bufs=2) as ostpool,
            tc.tile_pool(name="ps", bufs=1, space="PSUM") as pspool,
        ):
            # ---------------- constants ----------------
            w_qk_t = [cpool.tile([128, 8 * HD], F32R, name=f"wqk{i}", tag=f"wqk{i}") for i in range(8)]
            w_v_t = [cpool.tile([128, 4 * HD], F32R, name=f"wv{i}", tag=f"wv{i}") for i in range(8)]
            w_out_t = [cpool.tile([128, D], F32R, name=f"wout{i}", tag=f"wout{i}") for i in range(2)]
            rotP_t = cpool.tile([128, 128], F32R)
            ident_t = cpool.tile([128, 128], F32R)
            maskb_t = cpool.tile([128, 4, 512], F32R)

            for dd in range(8):
                nc.sync.dma_start(
                    w_qk_t[dd][:], w_qk[128 * dd : 128 * dd + 128, :].bitcast(F32R)
                )
            for dd in range(8):
                nc.gpsimd.dma_start(
                    w_v_t[dd][:], w_v[128 * dd : 128 * dd + 128, :].bitcast(F32R)
                )
            for kk in range(2):
                nc.gpsimd.dma_start(
                    w_out_t[kk][:], w_out[128 * kk : 128 * kk + 128, :].bitcast(F32R)
                )
            nc.sync.dma_start(rotP_t[:], rotP[:].bitcast(F32R))
            nc.gpsimd.dma_start(ident_t[:], ident[:].bitcast(F32R))
            nc.gpsimd.dma_start(
                maskb_t[:], maskb[:].bitcast(F32R).rearrange("p (o s) -> p o s", o=4)
            )

            # v_aug storage: per (t-block j, head h): [v_h(64) | 1] = 65 cols
            va_t = vapool.tile([128, NT, 4, HD + 1], F32R)
            nc.gpsimd.dma_start(
                va_t[:, :, :, HD : HD + 1],
                onesd[:].bitcast(F32R).rearrange("p (j h c) -> p j h c", j=NT, h=4),
            )

            # final rotated qT/kT: tiles [q_h0;q_h1], [k_h0;k_h1], [q_h2;q_h3], [k_h2;k_h3]
            qkT = [qkTpool.tile([128, S], F32R, name=f"qkT{i}", tag=f"qkT{i}") for i in range(4)]

            def load_proj(i):
                s_sl = slice(512 * i, 512 * i + 512)
                # ---- loads for this quarter ----
                xq = xqpool.tile([128, 8, 512], F32R, tag="xq", name="xq")
                for dd in range(8):
                    nc.sync.dma_start(
                        xq[:, dd, :],
                        xT[128 * dd : 128 * dd + 128, s_sl].bitcast(F32R),
                    )
                cos_q = trigpool.tile([128, 512], F32, tag="cos", name="cos_q")
                sin_q = trigpool.tile([128, 512], F32, tag="sin", name="sin_q")
                nc.sync.dma_start(cos_q[:], cos2[:, s_sl])
                nc.sync.dma_start(sin_q[:], sin2[:, s_sl])

                # ---- qk projection + rope for this quarter ----
                for mt in range(4):
                    ps = pspool.tile([128, 512], F32, tag="mm1", bufs=2, name="ps")
                    for dd in range(8):
                        nc.tensor.matmul(
                            ps[:],
                            w_qk_t[dd][:, 128 * mt : 128 * mt + 128],
                            xq[:, dd, :],
                            start=(dd == 0),
                            stop=(dd == 7),
                        )
                    qk_raw = qkrawpool.tile([128, 512], F32R, tag="qkraw", name="qk_raw")
                    nc.vector.tensor_copy(qk_raw[:], ps[:])
                    psr = pspool.tile([128, 512], F32, tag="mm1", bufs=2, name="psr")
                    nc.tensor.matmul(psr[:], rotP_t[:], qk_raw[:], start=True, stop=True)
                    rotsin = rspool.tile([128, 512], F32, tag="rs", name="rotsin")
                    nc.vector.tensor_mul(rotsin[:], psr[:], sin_q[:])
                    qkcos = rspool.tile([128, 512], F32, tag="qkcos", name="qkcos")
                    nc.gpsimd.tensor_mul(qkcos[:], qk_raw[:].bitcast(F32), cos_q[:])
                    nc.vector.tensor_add(qkT[mt][:, s_sl], qkcos[:], rotsin[:])

                # ---- v projection for this quarter ----
                for st in range(4):
                    j = 4 * i + st
                    psv = pspool.tile([128, 4 * HD], F32, tag="mm1", bufs=2, name="psv")
                    for dd in range(8):
                        nc.tensor.matmul(
                            psv[:],
                            xq[:, dd, 128 * st : 128 * st + 128],
                            w_v_t[dd][:],
                            start=(dd == 0),
                            stop=(dd == 7),
                        )
                    nc.vector.tensor_copy(
                        va_t[:, j, :, 0:HD], psv[:].rearrange("p (h c) -> p h c", h=4)
                    )

            def attention(i):
                s_sl = slice(512 * i, 512 * i + 512)
                nblk = 4 * i + 4 if causal else NT
                ao = [
                    aopool.tile([128, 512], F32R, tag="aot", name=f"ao{hp}")
                    for hp in range(2)
                ]
                for hp in range(2):
                    qt = qkT[2 * hp]
                    kt = qkT[2 * hp + 1]
                    ps_pv_e = pspool.tile([HD + 1, 512], F32, tag="pv", bufs=2, name="ps_pv_e")
                    ps_pv_o = pspool.tile([HD + 1, 512], F32, tag="pv", bufs=2, name="ps_pv_o")
                    for j in range(nblk):
                        dvr = causal and 4 * i <= j
                        o = j - 4 * i if dvr else 0
                        # columns s' < 128*o of a diagonal block are fully
                        # masked -> restrict compute to [128*o : 512].
                        lo = 128 * o
                        reg = slice(lo, 512)
                        # fp32r needs moving dim >= 256 for full rate; for
                        # o == 3 compute qk full-width (same cycles).
                        qk_lo = lo if lo <= 256 else 0
                        ps_qk = pspool.tile([128, 2, 512], F32, tag="mm2", bufs=2, name="ps_qk")
                        for sl2 in range(2):
                            hb = 64 * sl2
                            nc.tensor.matmul(
                                ps_qk[:, sl2, qk_lo:512],
                                kt[hb : hb + 64, 128 * j : 128 * j + 128],
                                qt[hb : hb + 64, 512 * i + qk_lo : 512 * i + 512],
                                start=True,
                                stop=not dvr,
                            )
                        if dvr:
                            for sl2 in range(2):
                                nc.tensor.matmul(
                                    ps_qk[:, sl2, reg],
                                    ident_t[:],
                                    maskb_t[:, o, reg],
                                    start=False,
                                    stop=True,
                                )
                        phat = phatpool.tile([128, 2, 512], F32R, tag="phat", name="phat")
                        nc.scalar.activation(
                            phat[:, :, reg],
                            ps_qk[:, :, reg],
                            AF.Exp,
                            scale=1.0 / math.sqrt(HD),
                        )
                        nc.tensor.matmul(
                            ps_pv_e[:, reg],
                            va_t[:, j, 2 * hp, :],
                            phat[:, 0, reg],
                            start=(j == 0),
                            stop=(j == nblk - 1),
                        )
                        nc.tensor.matmul(
                            ps_pv_o[:, reg],
                            va_t[:, j, 2 * hp + 1, :],
                            phat[:, 1, reg],
                            start=(j == 0),
                            stop=(j == nblk - 1),
                        )
                    for sl2, ps_pv in ((0, ps_pv_e), (1, ps_pv_o)):
                        hb = 64 * sl2
                        den = normpool.tile([1, 512], F32, tag="den", name="den")
                        rec1 = normpool.tile([1, 512], F32, tag="rec1", name="rec1")
                        rec = normpool.tile([64, 512], F32, tag="rec", name="rec")
                        nc.vector.tensor_copy(den[:], ps_pv[HD : HD + 1, :])
                        nc.vector.reciprocal_approx_fast(rec1[:], den[:])
                        nc.gpsimd.partition_broadcast(rec[:], rec1[:])
                        nc.vector.tensor_mul(
                            ao[hp][hb : hb + 64, :], ps_pv[0:HD, :], rec[:]
                        )
                return ao

            def outproj(i, ao):
                for st in range(4):
                    ssl = slice(512 * i + 128 * st, 512 * i + 128 * st + 128)
                    sloc = slice(128 * st, 128 * st + 128)
                    ostage = ostpool.tile([128, D], F32, tag="ost", name="ostage")
                    for n2 in range(2):
                        ps_o = pspool.tile([128, 512], F32, tag="mm2", bufs=2, name="ps_o")
                        for kk in range(2):
                            nc.tensor.matmul(
                                ps_o[:],
                                ao[kk][:, sloc],
                                w_out_t[kk][:, 512 * n2 : 512 * n2 + 512],
                                start=(kk == 0),
                                stop=(kk == 1),
                            )
                        nc.vector.tensor_copy(
                            ostage[:, 512 * n2 : 512 * n2 + 512], ps_o[:]
                        )
                    nc.scalar.dma_start(outp[ssl, :], ostage[:])

            # causal: fused per-chunk pipeline (attention chunk i only needs
            # kT/v for t-blocks <= chunk end). non-causal: attention needs the
            # full kT/v, so project everything first.
            for _rep in range(reps):
                if causal:
                    for i in range(NI):
                        load_proj(i)
                        outproj(i, attention(i))
                else:
                    for i in range(NI):
                        load_proj(i)
                    for i in range(NI):
                        outproj(i, attention(i))

    nc.compile()
    return nc


# --------------------------------------------------------------------------
# host-side: constants, sharding, assembly
# --------------------------------------------------------------------------

def _rope_tables():
    inv_freq = 1.0 / (
        ROPE_BASE ** (np.arange(0, HD, 2, dtype=np.float32) / HD)
    )
    positions = np.arange(S, dtype=np.float32)
    freqs = np.outer(positions, inv_freq).astype(np.float32)     # [S, 32]
    emb = np.concatenate((freqs, freqs), axis=-1)                # [S, 64]
    cosT = np.cos(emb).T.astype(np.float32)                      # [64, S]
    sinT = np.sin(emb).T.astype(np.float32)
    cos2 = np.vstack([cosT, cosT]).copy()                        # [128, S]
    sin2 = np.vstack([sinT, sinT]).copy()
    return cos2, sin2


def _rot_lhsT():
    # rotate_half (interleaved): rot[2i] = -x[2i+1], rot[2i+1] = x[2i]
    # P[j, i]: rot[j] = sum_i P[j, i] x[i]; lhsT[i, j] = P[j, i]
    P = np.zeros((128, 128), np.float32)
    for base in (0, 64):
        for i2 in range(HD // 2):
            P[base + 2 * i2, base + 2 * i2 + 1] = -1.0
            P[base + 2 * i2 + 1, base + 2 * i2] = 1.0
    return P.T.copy()


def _mask_bias():
    # maskb[t, o, s'] = NEG where (t + 128*o) > s'  (within a 512 s-chunk,
    # for the 4 diagonal t-blocks at offsets o = j - 4i)
    t = np.arange(128)[:, None, None]
    o = np.arange(4)[None, :, None]
    sp = np.arange(512)[None, None, :]
    mb = np.where(t + 128 * o > sp, np.float32(NEG), np.float32(0.0))
    return mb.reshape(128, 4 * 512).astype(np.float32)


_CACHE: dict = {}


def _get_nc(causal: bool):
    key = ("nc", causal)
    if key not in _CACHE:
        _CACHE[key] = build_nc(causal)
    return _CACHE[key]


def _classify_mask(mask: np.ndarray) -> str:
    m = np.asarray(mask).reshape(S, S)
    if np.array_equal(m != 0, np.tril(np.ones((S, S), bool))):
        return "causal"
    if np.all(m != 0):
        return "full"
    return "other"


def make_in_maps(x, w_qkv, w_out):
    """Build the 8 per-core input dicts."""
    cos2, sin2 = _rope_tables()
    rotP = _rot_lhsT()
    maskb = _mask_bias()
    ident = np.eye(128, dtype=np.float32)
    onesd = np.ones((128, 64), np.float32)

    w3 = np.asarray(w_qkv).reshape(D, 3, H, HD)   # [D, {q,k,v}, H, hd]
    wo = np.asarray(w_out)                        # [D, D]; rows indexed [h, hd]
    xT = [np.ascontiguousarray(np.asarray(x)[b].T) for b in range(B)]  # [D, S]

    in_maps = []
    for c in range(8):
        b, hg = divmod(c, 4)
        hs = [4 * hg + i for i in range(HPC)]
        # w_qk cols: [q_h0, q_h1, k_h0, k_h1, q_h2, q_h3, k_h2, k_h3]
        wqk_cols = []
        for pair in range(2):
            for t in range(2):  # 0 = q, 1 = k
                for hh in (hs[2 * pair], hs[2 * pair + 1]):
                    wqk_cols.append(w3[:, t, hh, :])
        w_qk_c = np.ascontiguousarray(np.concatenate(wqk_cols, axis=1))  # [D, 512]
        w_v_c = np.ascontiguousarray(
            np.concatenate([w3[:, 2, hh, :] for hh in hs], axis=1)
        )  # [D, 256]
        w_out_c = np.ascontiguousarray(
            np.concatenate([wo[HD * hh : HD * hh + HD, :] for hh in hs], axis=0)
        )  # [256, D]
        in_maps.append(
            {
                "xT": xT[b],
                "w_qk": w_qk_c,
                "w_v": w_v_c,
                "w_out": w_out_c,
                "cos2": cos2,
                "sin2": sin2,
                "rotP": rotP,
                "ident": ident,
                "maskb": maskb,
                "onesd": onesd,
            }
        )
    return in_maps


def _reference_numpy(x, mask, w_qkv, w_out):
    """Exact fallback for non-causal, non-full masks (slow, host-side)."""
    x = np.asarray(x, np.float32)
    qkv = (x @ w_qkv).reshape(B, S, 3, H, HD)
    qkv = np.transpose(qkv, (2, 0, 3, 1, 4))
    q, k, v = qkv[0], qkv[1], qkv[2]
    cos2, sin2 = _rope_tables()
    cos = cos2[:HD].T[None, None]
    sin = sin2[:HD].T[None, None]

    def rot(t):
        t1 = t[..., ::2]
        t2 = t[..., 1::2]
        return np.stack((-t2, t1), axis=-1).reshape(t.shape)

    q = q * cos + rot(q) * sin
    k = k * cos + rot(k) * sin
    attn = np.einsum("bhsd,bhtd->bhst", q, k) / math.sqrt(HD)
    m = np.asarray(mask).reshape(1, 1, S, S)
    attn = np.where(m == 0, -np.inf, attn)
    attn = attn - attn.max(-1, keepdims=True)
    np.exp(attn, out=attn)
    attn /= attn.sum(-1, keepdims=True)
    out = np.einsum("bhst,bhtd->bhsd", attn, v)
    out = np.transpose(out, (0, 2, 1, 3)).reshape(B, S, D)
    return (out @ w_out).astype(np.float32)


class Runner:
    """Cached jitted SPMD runner (mirrors bass2jax.run_bass_via_pjrt)."""

    def __init__(self, nc, n_cores: int = 8):
        import jax
        import concourse.mybir as _mybir
        from concourse import bass2jax
        from jax.experimental.shard_map import shard_map
        from jax.sharding import Mesh, PartitionSpec

        bass2jax.install_neuronx_cc_hook()
        self.jax = jax
        self.n_cores = n_cores
        self._nc = nc
        in_names, out_names, out_avals, zero_outs = [], [], [], []
        for alloc in nc.m.functions[0].allocations:
            if not isinstance(alloc, _mybir.MemoryLocationSet):
                continue
            name = alloc.memorylocations[0].name
            if alloc.kind == "ExternalInput":
                in_names.append(name)
            elif alloc.kind == "ExternalOutput":
                out_names.append(name)
                shape = tuple(alloc.tensor_shape)
                dtype = _mybir.dt.np(alloc.dtype)
                out_avals.append(jax.core.ShapedArray(shape, dtype))
                zero_outs.append(np.zeros(shape, dtype))
        self.in_names = list(in_names)
        self.out_names = out_names
        self.out_avals = out_avals
        self.zero_outs = zero_outs
        all_names = in_names + out_names

        def _body(*args):
            outs = bass2jax._bass_exec_p.bind(
                *args,
                out_avals=tuple(out_avals),
                in_names=tuple(all_names),
                out_names=tuple(out_names),
                lowering_input_output_aliases=(),
                sim_require_finite=True,
                sim_require_nnan=True,
                nc=nc,
            )
            return tuple(outs)

        devices = jax.devices()[:n_cores]
        self.mesh = Mesh(np.asarray(devices), ("core",))
        n_args = len(all_names)
        self.sharded = jax.jit(
            shard_map(
                _body,
                mesh=self.mesh,
                in_specs=(PartitionSpec("core"),) * n_args,
                out_specs=(PartitionSpec("core"),) * len(out_names),
                check_rep=False,
            )
        )

    def concat_inputs(self, in_maps):
        cols = []
        for name in self.in_names:
            if name == "partition_id":
                cols.append(
                    np.arange(self.n_cores, dtype=np.uint32).reshape(
                        self.n_cores, 1
                    )
                )
            else:
                cols.append(
                    np.concatenate([np.asarray(m[name]) for m in in_maps], axis=0)
                )
        return cols

    def device_put(self, concat_in):
        """Place concatenated inputs (and zero output buffers) on the mesh."""
        from jax.sharding import NamedSharding, PartitionSpec

        sh = NamedSharding(self.mesh, PartitionSpec("core"))
        args = concat_in + [
            np.zeros((self.n_cores * z.shape[0], *z.shape[1:]), z.dtype)
            for z in self.zero_outs
        ]
        return [self.jax.device_put(a, sh) for a in args]

    def run_dev(self, dev_args):
        return self.sharded(*dev_args)

    def make_bench(self, n_reps: int):
        """Jitted fn executing the NEFF n_reps times serially on-device."""
        import jax
        from concourse import bass2jax
        from jax.experimental.shard_map import shard_map
        from jax.sharding import Mesh, PartitionSpec

        nc = self._nc
        out_avals = self.out_avals
        all_names = self.in_names + self.out_names
        out_names = self.out_names

        def _body(*args):
            outs = None
            for _ in range(n_reps):
                outs = bass2jax._bass_exec_p.bind(
                    *args,
                    out_avals=tuple(out_avals),
                    in_names=tuple(all_names),
                    out_names=tuple(out_names),
                    lowering_input_output_aliases=(),
                    sim_require_finite=True,
                    sim_require_nnan=True,
                    nc=nc,
                )
            return tuple(outs)

        n_args = len(all_names)
        return jax.jit(
            shard_map(
                _body,
                mesh=self.mesh,
                in_specs=(PartitionSpec("core"),) * n_args,
                out_specs=(PartitionSpec("core"),) * len(out_names),
                check_rep=False,
            )
        )

    def run(self, in_maps):
        dev_args = self.device_put(self.concat_inputs(in_maps))
        out_arrs = self.sharded(*dev_args)
        outs = []
        for c in range(self.n_cores):
            outs.append(
                {
                    name: np.asarray(out_arrs[i]).reshape(
                        self.n_cores, *self.out_avals[i].shape
                    )[c]
                    for i, name in enumerate(self.out_names)
                }
            )
        return outs


def _get_runner(causal: bool) -> Runner:
    key = ("runner", causal)
    if key not in _CACHE:
        _CACHE[key] = Runner(_get_nc(causal))
    return _CACHE[key]


def run_spmd(in_maps, causal: bool = True, **kw):
    nc = _get_nc(causal)
    return run_bass_kernel_spmd(nc, in_maps, core_ids=list(range(8)), **kw)


def kernel(x, mask, w_qkv, w_out):
    kind = _classify_mask(mask)
    if kind == "other":
        return _reference_numpy(x, mask, w_qkv, w_out)
    in_maps = make_in_maps(x, w_qkv, w_out)
    res = run_spmd(in_maps, causal=(kind == "causal"))
    out = np.zeros((B, S, D), np.float32)
    for c in range(8):
        out[c // 4] += res.results[c]["outp"]
    return out


if __name__ == "__main__":
    rng = np.random.default_rng(0)
    x = rng.standard_normal((B, S, D)).astype(np.float32)
    mask = np.tril(np.ones((S, S), np.int32)).reshape(1, 1, S, S)
    w_qkv = (rng.standard_normal((D, 3 * D)) * 0.02).astype(np.float32)
    w_out = (rng.standard_normal((D, D)) * 0.02).astype(np.float32)
    got = kernel(x, mask, w_qkv, w_out)
    print("kernel ran, out shape", got.shape)
